# revision 9
# baseline (speedup 1.0000x reference)
"""AttnBlock (GroupNorm -> QKV 1x1 -> full NxN attention -> proj -> residual)
for Trainium2, SPMD over 8 NeuronCores.

Sharding: data-parallel over batch (2) x query-pixel blocks (4 of 1024 px).
Each core receives its batch image x [C, N] PERMUTED so that its own query
block occupies pixels [0, NQ): attention is permutation-invariant over keys,
so K/V may be computed in any pixel order as long as it is consistent.  K and
V^T are computed redundantly per batch pair, queries are disjoint.  No
collectives.

Key structural points (v4):
  - x and the QKV weights arrive as bf16 (host-converted); x is loaded ONCE
    into SBUF in 4 quarter DMAs and stays resident: GroupNorm stats, all
    matmuls, and the residual read it.  End-to-end rel err ~3e-3 (numpy
    model), gate is 2e-2.
  - The GroupNorm affine hn = A*x + B is folded into the weights: after the
    stats pass, wk/wv/wq are scaled in place by A (per input channel = per
    partition).  The B-terms: K's bias is constant over keys and cancels in
    softmax (dropped); Q's bias tq = Wq_s@B + bq_s is computed on device
    (tiny matvecs) and applied at Q production via ACT bias; V's bias
    tv = Wv@B + bv is constant over pixels, commutes through the softmax
    average, and folds into the proj bias: bp_dev = Wp@(Wv@B) + bpT.
    No per-pixel hn is ever materialized.
  - Attention: S^T[k,q] = K^T@Q (bf16) -> exp (ACT, no max subtraction:
    scores ~ N(0,1), fp32-safe) -> O[c,q] += V^T-block^T @ P accumulated in
    PSUM over k-tiles (lhsT = V^T block, so O lands directly in [c,q]
    layout: no transposes).  Softmax denominators: Pool-engine running sum
    of exp tiles (f32), ones-matmul folds partitions -> [1,QP], reciprocal,
    ones-broadcast matmul -> [128,QP].
  - proj runs on UNNORMALIZED O in f32r (division commutes through the
    linear proj); the epilogue divides, adds bias + residual, and ships one
    batched out DMA per query pass.
"""

from contextlib import ExitStack

import numpy as np

import concourse.bacc as bacc
import concourse.bass as bass
import concourse.mybir as mybir
import concourse.tile as tile

F32 = mybir.dt.float32
F32R = mybir.dt.float32r
BF16 = mybir.dt.bfloat16
AF = mybir.ActivationFunctionType


def build_program(C=512, G=32, N=4096, NQ=1024, eps=1e-5, precision="tf32"):
    """Emit the per-core Bass program (SPMD; per-core data differs only)."""
    P = 128
    CS = C // P                  # channel subtiles
    KT = N // P                  # key/pixel tiles
    NCH = 512                    # phase-2 production chunk / bn window (px)
    NCHUNKS = N // NCH
    QCHUNKS = NQ // NCH          # leading chunks that are also query pixels
    NW = 1024                    # x-load quarter width
    NWQ = N // NW
    QP = min(512, NQ)            # query-pass width
    QPASSES = NQ // QP
    cpg = C // G                 # channels per group
    GPS = P // cpg               # groups per channel-subtile
    assert C % P == 0 and N % P == 0 and NQ % QP == 0 and P % cpg == 0
    assert NQ % NCH == 0 and N % NW == 0
    MMDT = F32R if precision == "tf32" else F32
    SDT = BF16                   # storage dtype for x / w / K / V^T / Q / P

    nc = bacc.Bacc(None, target_bir_lowering=False)

    x_d = nc.dram_tensor("x", [C, N], SDT, kind="ExternalInput")
    wt_d = {
        w: nc.dram_tensor(f"w{w}t", [C, C], SDT, kind="ExternalInput")
        for w in ("q", "k", "v")
    }
    wpt_d = nc.dram_tensor("wpt", [C, C], MMDT, kind="ExternalInput")
    bqT_d = nc.dram_tensor("bqT", [P, CS], F32, kind="ExternalInput")
    bpT_d = nc.dram_tensor("bpT", [P, CS], F32, kind="ExternalInput")
    gamma_d = nc.dram_tensor("gamma", [C], F32, kind="ExternalInput")
    beta_d = nc.dram_tensor("beta", [C], F32, kind="ExternalInput")
    indg_d = nc.dram_tensor("indg", [P, GPS], F32, kind="ExternalInput")
    inde_d = nc.dram_tensor("inde", [GPS, P], F32, kind="ExternalInput")
    out_d = nc.dram_tensor("out", [C, NQ], F32, kind="ExternalOutput")

    x_r = x_d[:, :].rearrange("(s p) n -> p s n", p=P)
    out_r = out_d[:, :].rearrange("(s p) n -> p s n", p=P)

    with tile.TileContext(nc) as tc, ExitStack() as st:
        const = st.enter_context(tc.tile_pool(name="const", bufs=1))
        big = st.enter_context(tc.tile_pool(name="big", bufs=1))
        small = st.enter_context(tc.tile_pool(name="small", bufs=1))

        # resident big tensors
        x_sb = big.tile([P, CS, N], SDT, tag="x")        # x, loaded once
        K_sb = big.tile([P, CS, N], SDT, tag="K")        # K[co, n]
        VT_sb = big.tile([P, KT, C], SDT, tag="VT")      # V^T[n, co]
        Q_sb = big.tile([P, CS, NQ], SDT, tag="Q")       # Q[co, nq] (scaled)
        wpT = big.tile([P, CS, C], MMDT, tag="wpT")      # proj weight

        # ---- x first: stats are the critical path -------------------------
        dma_engs = [nc.sync, nc.gpsimd]
        for qd in range(NWQ):
            dma_engs[qd % len(dma_engs)].dma_start(
                out=x_sb[:, :, qd * NW:(qd + 1) * NW],
                in_=x_r[:, :, qd * NW:(qd + 1) * NW])

        # ---- weights + constants on the scalar queue ----------------------
        with ExitStack() as st1:
            wqkv = st1.enter_context(tc.tile_pool(name="wqkv", bufs=3))
            ps_a = st1.enter_context(tc.tile_pool(name="ps_a", bufs=3,
                                                  space="PSUM"))
            ps_mv = st1.enter_context(tc.tile_pool(name="ps_mv", bufs=2,
                                                   space="PSUM"))

            wk = wqkv.tile([P, CS, C], SDT, tag="wt", name="w_k")
            nc.scalar.dma_start(
                out=wk, in_=wt_d["k"][:, :].rearrange("(s p) c -> p s c", p=P))
            indg = const.tile([P, GPS], F32, tag="indg")
            nc.scalar.dma_start(out=indg, in_=indg_d[:, :])
            inde = const.tile([GPS, P], F32, tag="inde")
            nc.scalar.dma_start(out=inde, in_=inde_d[:, :])
            gammaT = const.tile([P, CS], F32, tag="gammaT")
            nc.scalar.dma_start(out=gammaT,
                                in_=gamma_d[:].rearrange("(s p) -> p s", p=P))
            betaT = const.tile([P, CS], F32, tag="betaT")
            nc.scalar.dma_start(out=betaT,
                                in_=beta_d[:].rearrange("(s p) -> p s", p=P))
            bqT = const.tile([P, CS], F32, tag="bqT")
            nc.scalar.dma_start(out=bqT, in_=bqT_d[:, :])
            bpT = const.tile([P, CS], F32, tag="bpT")
            nc.scalar.dma_start(out=bpT, in_=bpT_d[:, :])
            wv = wqkv.tile([P, CS, C], SDT, tag="wt", name="w_v")
            nc.scalar.dma_start(
                out=wv, in_=wt_d["v"][:, :].rearrange("(s p) c -> p s c", p=P))
            wq = wqkv.tile([P, CS, C], SDT, tag="wt", name="w_q")
            nc.scalar.dma_start(
                out=wq, in_=wt_d["q"][:, :].rearrange("(s p) c -> p s c", p=P))
            nc.scalar.dma_start(
                out=wpT, in_=wpt_d[:, :].rearrange("(s p) c -> p s c", p=P))

            ones_c = const.tile([P, 1], F32, tag="ones_c")
            nc.vector.memset(ones_c, 1.0)
            ones_r = const.tile([1, P], F32, tag="ones_r")
            nc.vector.memset(ones_r, 1.0)
            eps_t = const.tile([P, 1], F32, tag="eps")
            nc.vector.memset(eps_t, eps)
            # ACT table preload off the critical path (Sqrt: stats tail;
            # Exp: first attention tile)
            dume = small.tile([P, 1], F32, tag="dume")
            nc.scalar.activation(out=dume, in_=eps_t, func=AF.Sqrt)
            nc.scalar.activation(out=dume, in_=eps_t, func=AF.Exp)

            # ---- phase 1: GroupNorm stats over resident x -----------------
            stats_all = small.tile([P, CS, NCHUNKS, 6], F32, tag="stats")
            for w0 in range(NCHUNKS):
                for s in range(CS):
                    nc.vector.bn_stats(out=stats_all[:, s, w0, :],
                                       in_=x_sb[:, s, w0 * NCH:(w0 + 1) * NCH])
            mv = small.tile([P, CS, 2], F32, tag="mv")
            for s in range(CS):
                nc.vector.bn_aggr(out=mv[:, s, :], in_=stats_all[:, s, :, :])

            # per-channel mean / E[x^2] -> group reduce via indicator matmul
            rhs8 = small.tile([P, 2 * CS], F32, tag="rhs8")
            nc.vector.tensor_copy(out=rhs8[:, 0:CS], in_=mv[:, :, 0])
            nc.vector.tensor_mul(out=rhs8[:, CS:], in0=mv[:, :, 0], in1=mv[:, :, 0])
            nc.vector.tensor_add(out=rhs8[:, CS:], in0=rhs8[:, CS:], in1=mv[:, :, 1])
            ps_g = ps_a.tile([GPS, 2 * CS], F32, tag="abank")
            nc.tensor.matmul(ps_g, lhsT=indg, rhs=rhs8, start=True, stop=True)
            gtmp = small.tile([GPS, 2 * CS], F32, tag="gtmp")
            nc.vector.tensor_scalar_mul(gtmp, ps_g, 1.0 / cpg)
            # gvar = E[x^2] - mean^2 ; grstd = 1/sqrt(gvar + eps)
            gsq = small.tile([GPS, CS], F32, tag="gsq")
            nc.vector.tensor_mul(out=gsq, in0=gtmp[:, 0:CS], in1=gtmp[:, 0:CS])
            e8 = small.tile([GPS, 2 * CS], F32, tag="e8")
            nc.vector.tensor_sub(out=e8[:, 0:CS], in0=gtmp[:, CS:], in1=gsq)
            nc.scalar.activation(out=e8[:, 0:CS], in_=e8[:, 0:CS], func=AF.Sqrt,
                                 bias=eps_t[:GPS], scale=1.0)
            nc.vector.reciprocal(out=e8[:, 0:CS], in_=e8[:, 0:CS])
            nc.vector.tensor_copy(out=e8[:, CS:], in_=gtmp[:, 0:CS])
            # expand groups -> channels
            ps_e = ps_a.tile([P, 2 * CS], F32, tag="abank")
            nc.tensor.matmul(ps_e, lhsT=inde, rhs=e8, start=True, stop=True)
            A_sb = small.tile([P, CS], F32, tag="A")     # A = gamma * rstd
            nc.vector.tensor_mul(out=A_sb, in0=ps_e[:, 0:CS], in1=gammaT)
            B_sb = small.tile([P, CS], SDT, tag="B")     # B = beta - A*mean
            nc.vector.tensor_mul(out=B_sb, in0=ps_e[:, CS:], in1=A_sb)
            nc.vector.tensor_sub(out=B_sb, in0=betaT, in1=B_sb)

            # ---- phase 2: K / V^T / Q from resident x (weights A-folded) --
            tq_sb = small.tile([P, CS], F32, tag="tq")
            u_sb = small.tile([P, CS], F32, tag="u")
            bp_dev = small.tile([P, CS], F32, tag="bp")

            for ch in range(NCHUNKS):
                c0 = ch * NCH
                if ch == 0:
                    # fold A into wk (per input channel = per partition)
                    for s in range(CS):
                        nc.vector.tensor_scalar_mul(
                            wk[:, s, :], wk[:, s, :], A_sb[:, s:s + 1])
                for cs in range(CS):     # K rows [co-sub, chunk] (bias drops)
                    ps_k = ps_a.tile([P, NCH], F32, tag="abank")
                    for s in range(CS):
                        nc.tensor.matmul(
                            ps_k, lhsT=wk[:, s, cs * P:(cs + 1) * P],
                            rhs=x_sb[:, s, c0:c0 + NCH],
                            start=(s == 0), stop=(s == CS - 1),
                        )
                    nc.scalar.activation(
                        out=K_sb[:, cs, c0:c0 + NCH], in_=ps_k,
                        func=AF.Identity, scale=1.0,
                    )
                if ch == 0:
                    # tq = Wq_s @ B + bq_s (RAW wq: A-scaling comes after)
                    ps_t = ps_mv.tile([P, CS], F32, tag="mv", name="ps_tq")
                    for cs in range(CS):
                        for s in range(CS):
                            nc.tensor.matmul(
                                ps_t[:, cs:cs + 1],
                                lhsT=wq[:, s, cs * P:(cs + 1) * P],
                                rhs=B_sb[:, s:s + 1],
                                start=(s == 0), stop=(s == CS - 1),
                                skip_group_check=True,
                            )
                    nc.vector.tensor_add(out=tq_sb, in0=ps_t, in1=bqT)
                    # u = Wv @ B (RAW wv)
                    ps_u = ps_mv.tile([P, CS], F32, tag="mv", name="ps_u")
                    for cs in range(CS):
                        for s in range(CS):
                            nc.tensor.matmul(
                                ps_u[:, cs:cs + 1],
                                lhsT=wv[:, s, cs * P:(cs + 1) * P],
                                rhs=B_sb[:, s:s + 1],
                                start=(s == 0), stop=(s == CS - 1),
                                skip_group_check=True,
                            )
                    nc.vector.tensor_copy(out=u_sb, in_=ps_u)
                    # now fold A into wv (Pool) and wq (DVE)
                    for s in range(CS):
                        nc.gpsimd.tensor_scalar_mul(
                            wv[:, s, :], wv[:, s, :], A_sb[:, s:s + 1])
                    for s in range(CS):
                        nc.vector.tensor_scalar_mul(
                            wq[:, s, :], wq[:, s, :], A_sb[:, s:s + 1])
                for ns in range(NCH // P):   # V^T rows [pixel-sub, all co]
                    ps_v = ps_a.tile([P, C], F32, tag="abank")
                    for s in range(CS):
                        nc.tensor.matmul(
                            ps_v, lhsT=x_sb[:, s, c0 + ns * P:c0 + (ns + 1) * P],
                            rhs=wv[:, s, :],
                            start=(s == 0), stop=(s == CS - 1),
                        )
                    nc.vector.tensor_copy(
                        out=VT_sb[:, ch * (NCH // P) + ns, :], in_=ps_v
                    )
                if ch < QCHUNKS:             # Q rows (own block = chunks 0..)
                    for cs in range(CS):
                        ps_q = ps_a.tile([P, NCH], F32, tag="abank")
                        for s in range(CS):
                            nc.tensor.matmul(
                                ps_q, lhsT=wq[:, s, cs * P:(cs + 1) * P],
                                rhs=x_sb[:, s, c0:c0 + NCH],
                                start=(s == 0), stop=(s == CS - 1),
                            )
                        nc.scalar.activation(
                            out=Q_sb[:, cs, c0:c0 + NCH], in_=ps_q,
                            func=AF.Identity, bias=tq_sb[:, cs:cs + 1],
                            scale=1.0,
                        )
                if ch == 1:
                    # bp_dev = Wp @ u + bpT  (tv = Wv@B + bv commutes through
                    # the softmax average into the proj bias; bv part is in
                    # host bpT already)
                    ps_z = ps_mv.tile([P, CS], F32, tag="mv", name="ps_z")
                    for cs in range(CS):
                        for s in range(CS):
                            nc.tensor.matmul(
                                ps_z[:, cs:cs + 1],
                                lhsT=wpT.bitcast(F32)[:, s, cs * P:(cs + 1) * P],
                                rhs=u_sb[:, s:s + 1],
                                start=(s == 0), stop=(s == CS - 1),
                                skip_group_check=True,
                            )
                    nc.vector.tensor_add(out=bp_dev, in0=ps_z, in1=bpT)

        # ---- phase 3: attention + proj + residual, per query pass ---------
        with ExitStack() as st2:
            ptp = st2.enter_context(tc.tile_pool(name="ptp", bufs=3))
            ocq = st2.enter_context(tc.tile_pool(name="ocq", bufs=1))
            outp = st2.enter_context(tc.tile_pool(name="outp", bufs=2))
            sm2 = st2.enter_context(tc.tile_pool(name="sm2", bufs=2))
            ps_s = st2.enter_context(tc.tile_pool(name="ps_s", bufs=2,
                                                  space="PSUM"))
            ps_o = st2.enter_context(tc.tile_pool(name="ps_o", bufs=CS,
                                                  space="PSUM"))
            ps_den = st2.enter_context(tc.tile_pool(name="ps_den", bufs=1,
                                                    space="PSUM"))
            ps_p = st2.enter_context(tc.tile_pool(name="ps_p", bufs=1,
                                                  space="PSUM"))

            for qp in range(QPASSES):
                q0 = qp * QP
                o_ps = []
                for _cs in range(CS):
                    o_tile = ps_o.tile([P, QP], F32, tag="o",
                                       name=f"o_{qp}_{_cs}")
                    o_ps.append(o_tile)
                acc = sm2.tile([P, QP], F32, tag="acc")
                pt_q = []

                def emit_s(kt):
                    s_ps = ps_s.tile([P, QP], F32, tag="sbank",
                                     name=f"s_ps_{qp}_{kt}")
                    for s in range(CS):
                        nc.tensor.matmul(
                            s_ps, lhsT=K_sb[:, s, kt * P:(kt + 1) * P],
                            rhs=Q_sb[:, s, q0:q0 + QP],
                            start=(s == 0), stop=(s == CS - 1),
                        )
                    pt = ptp.tile([P, QP], SDT, tag="pt",
                                  name=f"pt_{qp}_{kt}")
                    nc.scalar.activation(out=pt, in_=s_ps, func=AF.Exp)
                    pt_q.append((kt, pt))

                emit_s(0)
                for kt in range(KT):
                    if kt + 1 < KT:
                        emit_s(kt + 1)
                    k0, pt = pt_q.pop(0)
                    assert k0 == kt
                    # Pool-engine softmax-denominator accumulator (bf16 in,
                    # f32 accum)
                    if kt == 0:
                        nc.gpsimd.tensor_copy(out=acc, in_=pt)
                    else:
                        nc.gpsimd.tensor_add(out=acc, in0=acc, in1=pt)
                    last = kt == KT - 1
                    for cs in range(CS):     # O[c,q] += V^T-block^T @ P
                        nc.tensor.matmul(
                            o_ps[cs], lhsT=VT_sb[:, kt, cs * P:(cs + 1) * P],
                            rhs=pt,
                            start=(kt == 0), stop=last,
                        )
                # drain O unnormalized (softmax division commutes through the
                # linear proj: out = (Wp@O)/den + bp + x)
                oc = ocq.tile([P, CS, QP], MMDT, tag="ocq")
                for cs in range(CS):
                    nc.vector.tensor_copy(out=oc[:, cs, :], in_=o_ps[cs])
                # denom: fold partitions -> [1, QP]; reciprocal; broadcast
                den_ps = ps_den.tile([1, QP], F32, tag="den",
                                     name=f"den_{qp}")
                nc.tensor.matmul(den_ps, lhsT=ones_c, rhs=acc,
                                 start=True, stop=True)
                rec = sm2.tile([1, QP], F32, tag="rec")
                nc.vector.reciprocal(out=rec, in_=den_ps)
                bc_ps = ps_den.tile([P, QP], F32, tag="den",
                                    name=f"bc_{qp}")
                nc.tensor.matmul(bc_ps, lhsT=ones_r, rhs=rec,
                                 start=True, stop=True)
                rec_bc = sm2.tile([P, QP], F32, tag="recbc")
                nc.vector.tensor_copy(out=rec_bc, in_=bc_ps)

                ot = outp.tile([P, CS, QP], F32, tag="ot")
                tt = outp.tile([P, CS, QP], F32, tag="tt")
                for cs in range(CS):          # proj rows [co-sub, qpass]
                    ps_pp = ps_p.tile([P, QP], F32, tag="pbank")
                    for s in range(CS):
                        nc.tensor.matmul(
                            ps_pp, lhsT=wpT[:, s, cs * P:(cs + 1) * P],
                            rhs=oc[:, s, :],
                            start=(s == 0), stop=(s == CS - 1),
                        )
                    nc.vector.tensor_mul(out=tt[:, cs, :], in0=ps_pp,
                                         in1=rec_bc)
                    nc.vector.scalar_tensor_tensor(
                        out=ot[:, cs, :], in0=tt[:, cs, :],
                        scalar=bp_dev[:, cs:cs + 1],
                        in1=x_sb[:, cs, q0:q0 + QP],
                        op0=mybir.AluOpType.add, op1=mybir.AluOpType.add,
                    )
                nc.sync.dma_start(out=out_r[:, :, q0:q0 + QP], in_=ot)

    nc.finalize()
    return nc


def make_consts(P=128, cpg=16):
    GPS = P // cpg
    indg = np.zeros((P, GPS), np.float32)
    for p in range(P):
        indg[p, p // cpg] = 1.0
    inde = indg.T.copy()
    return {
        "indg": indg,
        "inde": inde,
    }


_PROGRAM_CACHE = {}


def _get_program(C, G, N, NQ, precision="tf32"):
    key = (C, G, N, NQ, precision)
    if key not in _PROGRAM_CACHE:
        _PROGRAM_CACHE[key] = build_program(C=C, G=G, N=N, NQ=NQ,
                                            precision=precision)
    return _PROGRAM_CACHE[key]


def make_in_maps(x, gn_w, gn_b, q_w, q_b, k_w, k_b, v_w, v_b, proj_w, proj_b,
                 n_cores=8, G=32):
    """Shard full inputs into per-core input maps (biases folded on host).

    Per-core x is pixel-permuted so the core's query block is first; attention
    is permutation-invariant over keys so K/V stay consistent.  x and the
    QKV weights ship as bf16.
    """
    import ml_dtypes
    bf = ml_dtypes.bfloat16
    f = lambda a: np.ascontiguousarray(np.asarray(a, dtype=np.float32))
    x = f(x)
    b, c, h, w = x.shape
    n = h * w
    qblocks = n_cores // b
    nq = n // qblocks
    cs = c // 128
    scale = np.float32(c ** -0.5)
    xf = x.reshape(b, c, n)

    def to_pcs(v):                       # [C] -> [128, CS] (c = 128*s + p)
        return np.ascontiguousarray(np.asarray(v, np.float32).reshape(cs, 128).T)

    common = {
        "wqt": np.ascontiguousarray((f(q_w).T * scale).astype(bf)),
        "wkt": np.ascontiguousarray(f(k_w).T.astype(bf)),
        "wvt": np.ascontiguousarray(f(v_w).T.astype(bf)),
        "wpt": f(proj_w).T.copy(),
        "bqT": to_pcs(f(q_b) * scale),
        "bpT": to_pcs(f(proj_w) @ f(v_b) + f(proj_b)),
        "gamma": f(gn_w), "beta": f(gn_b),
        **make_consts(cpg=c // G),
    }
    in_maps = []
    for i in range(n_cores):
        bi, qi = divmod(i, qblocks)
        xb = xf[bi]
        qs, qe = qi * nq, (qi + 1) * nq
        xperm = np.concatenate([xb[:, qs:qe], xb[:, :qs], xb[:, qe:]], axis=1)
        in_maps.append({
            **common,
            "x": np.ascontiguousarray(xperm.astype(bf)),
        })
    return in_maps, (b, c, h, w, n, nq, qblocks)


def kernel(x, gn_w, gn_b, q_w, q_b, k_w, k_b, v_w, v_b, proj_w, proj_b):
    from concourse.bass_utils import run_bass_kernel_spmd

    in_maps, (b, c, h, w, n, nq, qblocks) = make_in_maps(
        x, gn_w, gn_b, q_w, q_b, k_w, k_b, v_w, v_b, proj_w, proj_b
    )
    n_cores = 8
    nc = _get_program(C=c, G=32, N=n, NQ=nq)
    res = run_bass_kernel_spmd(nc, in_maps, list(range(n_cores))).results
    out = np.empty((b, c, n), np.float32)
    for i in range(n_cores):
        bi, qi = divmod(i, qblocks)
        out[bi, :, qi * nq:(qi + 1) * nq] = res[i]["out"]
    return out.reshape(b, c, h, w)
